# revision 41
# baseline (speedup 1.0000x reference)
"""Trainium2 Bass kernel for per-token outer-product softmax attention.

Reference computation (per token t of 1600, H=256):
    k = tanh(x W0 + b0);  q = tanh(x W1 + b1)
    scores[i,j] = k[i]*q[j];  attn = softmax_j(scores);  out = attn @ x

Key algebra: k,q are tanh outputs so k[i]*q[j] in (-1,1). On [-1,1],
exp(s) is approximated well below the 2e-2 output tolerance by a
low-degree minimax polynomial P(s) = sum_d c_d s^d, and P(k_i q_j) =
sum_d c_d k_i^d q_j^d is SEPARABLE. Softmax numerator/denominator become
per-token moments:
    num_i = sum_d (c_d sum_j q_j^d x_j) k_i^d
    den_i = sum_d (c_d sum_j q_j^d)     k_i^d
so the 256x256 scores tensor is never materialized. Moments come free as
accum_out of the product ops (coefficients folded into the op scalars,
m0 via a tiny PE ones-matmul); both k-polynomials are evaluated as
Horner chains of fused DVE scalar_tensor_tensor steps (no k-power tiles
needed). Working dtype is fp16 (output fp32): end-to-end rel-L2 err
~2.9e-3 at D=3, tolerance 2e-2.

Sharding: pure data parallel over tokens, 200 tokens/core x 8 cores;
weights replicated.
"""

import numpy as np
from contextlib import ExitStack

import concourse.bass as bass
import concourse.bacc as bacc
import concourse.tile as tile
from concourse import mybir
from concourse.bass_utils import run_bass_kernel_spmd

F32 = mybir.dt.float32
F16 = mybir.dt.float16
AF = mybir.ActivationFunctionType
OP = mybir.AluOpType

B, S, M, H = 4, 10, 40, 256
T = B * S * M            # 1600 tokens
NCORES = 8
TC = T // NCORES         # 200 tokens per core
BLOCKS = [(0, 128), (128, TC - 128)]

# Minimax-relative-error coefficients (monomial basis) of exp on [-1,1].
# Poly max rel err: D=3 -> 5.0e-3, D=4 -> 5.0e-4; end-to-end output
# rel-L2 err: D=3 fp16 ~3e-3, D=4 fp32 2.9e-4 (tolerance 2e-2).
COEFS = {
    3: [0.99650635, 1.0107962638, 0.5388581246, 0.1585305384],
    4: [0.9996280079, 0.9979377479, 0.5028966853, 0.1764876527,
        0.0399652955],
}

D = 3

# Placement/config knobs (tuned via CoreSim + HW loop benchmarks).
CFG = {
    "dtype": "f16",           # working dtype for powers/chains/products
    "q2": "act",              # act (Square, free s2 accum) | pool | dve
    "k2": "pool",             # act | pool | dve
    "q3": "pool",             # pool | dve
    "k3": "pool",
    "s1": "free",             # free (tanh accum + scale TT) | dve (TS-acc)
    "s2": "free",             # free (q2==act) | dve (TS-acc) | act (Id-acc)
    "s3": "dve",              # dve | act
    "m0": "pe",               # pe (ones-matmul) | dve | act
    # m1..mD: "ttr" (DVE tensor_tensor_reduce, 1 op) | "stt" (DVE fused)
    #         | "pool_dve" (Pool product + DVE TS-acc)
    #         | "pool_act" (Pool product + ACT Id-acc)
    # NOTE: "ttr" (tensor_tensor_reduce) crashes on HW — do not use.
    "m": ["stt", "stt", "pool_act", "stt"][: D],
    "uD_bias_act": True,      # uD final +aD0 on ACT (Identity bias)
    # chain form: "power" (ascending power basis, needs K2/K3) or
    # "horner" (descending, all-DVE TS+STT, no K powers needed)
    "uN_form": "horner",
    "uD_form": "horner",
    # power-form steps 1..D: step1: "dve" (TS) | "act" (Id scale+bias);
    # steps>=2: "stt" (DVE fused) | "ts_pool" (DVE TS + Pool add)
    #           | "ts_dve" (DVE TS + DVE add)
    "uN": ["dve", "stt", "stt", "stt"][: D],
    "uD": ["dve", "stt", "stt", "stt"][: D],
    "final": "ts_pool",       # stt (DVE) | ts_pool (DVE TS + Pool mult)
    "interleave": True,       # emit b0 head, b1 head, b0 tail, b1 tail
    "w_dma": "gpsimd",        # queue for weight DMAs: sync | scalar | gpsimd
    "x_dma": "sync",
    "out_dma": "sync",
    "io_bufs": 4,
    "work_bufs": 4,
    "pows_bufs": 4,
    "mom_bufs": 4,
    "ps_bufs": 3,
    "scrp_bufs": 16,
}


def build_kernel(reps: int = 1, with_bias: bool = True) -> bass.Bass:
    coef = COEFS[D]
    FW = F16 if CFG["dtype"] == "f16" else F32
    # raw den-moment columns needing the cden scale TT
    raw_ds = [d for d, k in [(1, "s1"), (2, "s2")] if CFG[k] == "free"]
    ncden = len(raw_ds)
    # col layout in A2d: raw cols first, then direct-scaled cols
    dcol = {}
    for i, d in enumerate(raw_ds):
        dcol[d] = i
    nxt = ncden
    for d in range(1, D + 1):
        if d not in dcol:
            dcol[d] = nxt
            nxt += 1

    WW = 2 * H + H + 2 * H + H   # [W1lo|W1hi|biasQ || W0lo|W0hi|biasK] fp16
    nc = bacc.Bacc("TRN2", target_bir_lowering=False, debug=False)
    xs = nc.declare_dram_parameter("xs", [TC, H], FW, isOutput=False)
    xst = nc.declare_dram_parameter("xst", [128, 2, TC], FW, isOutput=False)
    wcat = nc.declare_dram_parameter("wcat", [128, WW], FW, isOutput=False)
    cdn = nc.declare_dram_parameter("cdn", [128, max(ncden, 1)], F32, isOutput=False)
    out = nc.declare_dram_parameter("out", [TC, H], F32, isOutput=True)

    with tile.TileContext(nc) as tc, ExitStack() as ctx:
        consts = ctx.enter_context(tc.tile_pool(name="consts", bufs=1))
        io = ctx.enter_context(tc.tile_pool(name="io", bufs=CFG["io_bufs"]))
        work = ctx.enter_context(tc.tile_pool(name="work", bufs=CFG["work_bufs"]))
        pows = ctx.enter_context(tc.tile_pool(name="pows", bufs=CFG["pows_bufs"]))
        scrp = ctx.enter_context(tc.tile_pool(name="scrp", bufs=CFG["scrp_bufs"]))
        mom = ctx.enter_context(tc.tile_pool(name="mom", bufs=CFG["mom_bufs"]))
        psKQ = ctx.enter_context(
            tc.tile_pool(name="psKQ", bufs=CFG["ps_bufs"], space="PSUM")
        )

        eng = {"dve": nc.vector, "pool": nc.gpsimd, "act": nc.scalar}
        x_eng = getattr(nc, CFG["x_dma"])
        out_eng = getattr(nc, CFG["out_dma"])

        ones1 = consts.tile([1, 128], FW)
        if with_bias:
            nc.gpsimd.memset(ones1, 1.0)
        if CFG["m0"] == "pe":
            c0col = consts.tile([128, 1], FW)
            nc.gpsimd.memset(c0col, float(coef[0]))
            psM = ctx.enter_context(tc.tile_pool(name="psM", bufs=2, space="PSUM"))
        aD0c = consts.tile([128, 1], F32)
        nc.gpsimd.memset(aD0c, float(coef[0]) * float(H))
        Xs = []
        XTs = []
        for t0, tl in BLOCKS:
            X = io.tile([128, H], FW, tag=f"X{t0}")
            x_eng.dma_start(out=X[:tl, :], in_=xs[t0 : t0 + tl, :])
            Xs.append(X)
            xT = io.tile([128, 2, 128], FW, tag=f"XT{t0}")
            nc.gpsimd.dma_start(out=xT[:, :, :tl], in_=xst[:, :, t0 : t0 + tl])
            XTs.append(xT)
        w_eng = getattr(nc, CFG["w_dma"])
        wallQ = consts.tile([128, 3 * H], FW)
        w_eng.dma_start(out=wallQ, in_=wcat[:, 0 : 3 * H])
        wallK = consts.tile([128, 3 * H], FW)
        w_eng.dma_start(out=wallK, in_=wcat[:, 3 * H : 6 * H])
        cden = consts.tile([128, max(ncden, 1)], F32)
        w_eng.dma_start(out=cden, in_=cdn[:, :])
        bsbQ = wallQ[0:1, 2 * H : 3 * H]
        bsbK = wallK[0:1, 2 * H : 3 * H]
        aD0 = float(coef[0]) * float(H)

        def head(bi):
            t0, tl = BLOCKS[bi]
            if True:
                X = Xs[bi]
                xT = XTs[bi]

                # ---- matmuls: queries first (moments only need Q and X)
                psQ = psKQ.tile([128, H], F32, tag="psQ")
                if with_bias:
                    nc.tensor.matmul(
                        psQ[:tl, :], ones1[:, :tl], bsbQ, start=True, stop=False
                    )
                nc.tensor.matmul(
                    psQ[:tl, :], xT[:, 0, :tl], wallQ[:, 0:256],
                    start=not with_bias, stop=False,
                )
                nc.tensor.matmul(
                    psQ[:tl, :], xT[:, 1, :tl], wallQ[:, 256:512],
                    start=False, stop=True,
                )
                psK = psKQ.tile([128, H], F32, tag="psK")
                if with_bias:
                    nc.tensor.matmul(
                        psK[:tl, :], ones1[:, :tl], bsbK, start=True, stop=False
                    )
                nc.tensor.matmul(
                    psK[:tl, :], xT[:, 0, :tl], wallK[:, 0:256],
                    start=not with_bias, stop=False,
                )
                nc.tensor.matmul(
                    psK[:tl, :], xT[:, 1, :tl], wallK[:, 256:512],
                    start=False, stop=True,
                )

                # A2n[:, d] = c_d * m_d ; A2d[:, dcol[d]] = c_d * s_d
                A2n = mom.tile([128, D + 1], F32, tag="A2n")
                SmD = mom.tile([128, max(ncden, 1)], F32, tag="SmD")
                A2d = mom.tile([128, D], F32, tag="A2d")

                # ---- aN0 = c0 * sum_j x
                if CFG["m0"] == "pe":
                    psM0 = psM.tile([128, 1], F32, tag="psM0")
                    nc.tensor.matmul(
                        psM0[:tl, :], xT[:, 0, :tl], c0col,
                        start=True, stop=False,
                    )
                    nc.tensor.matmul(
                        psM0[:tl, :], xT[:, 1, :tl], c0col,
                        start=False, stop=True,
                    )
                    aN0 = psM0[:tl, 0:1]
                else:
                    j0 = scrp.tile([128, H], FW, tag="scr")
                    if CFG["m0"] == "act":
                        nc.scalar.activation(
                            j0[:tl, :], X[:tl, :], AF.Identity,
                            scale=float(coef[0]), accum_out=A2n[:tl, 0:1],
                        )
                    else:
                        nc.vector.tensor_scalar(
                            out=j0[:tl, :], in0=X[:tl, :], scalar1=float(coef[0]),
                            scalar2=0.0, op0=OP.mult, op1=OP.add,
                            accum_out=A2n[:tl, 0:1],
                        )
                    aN0 = A2n[:tl, 0:1]

                # ---- tanh (accum gives raw s1 when s1 == "free")
                Qt = work.tile([128, H], FW, tag="Qt")
                kw1 = (
                    {"accum_out": SmD[:tl, dcol[1] : dcol[1] + 1]}
                    if CFG["s1"] == "free" else {}
                )
                nc.scalar.activation(Qt[:tl, :], psQ[:tl, :], AF.Tanh, **kw1)
                Q = Qt[:tl, :]
                Kt = work.tile([128, H], FW, tag="Kt")
                nc.scalar.activation(Kt[:tl, :], psK[:tl, :], AF.Tanh)
                K = Kt[:tl, :]

                # ---- powers
                Q2 = pows.tile([128, H], FW, tag="Q2")
                if CFG["q2"] == "act":
                    kw = (
                        {"accum_out": SmD[:tl, dcol[2] : dcol[2] + 1]}
                        if CFG["s2"] == "free" else {}
                    )
                    nc.scalar.activation(Q2[:tl, :], Q, AF.Square, **kw)
                else:
                    eng[CFG["q2"]].tensor_mul(Q2[:tl, :], Q, Q)
                need_kp = "power" in (CFG["uN_form"], CFG["uD_form"])
                KP = {1: K}
                if need_kp:
                    K2 = pows.tile([128, H], FW, tag="K2")
                    if CFG["k2"] == "act":
                        nc.scalar.activation(K2[:tl, :], K, AF.Square)
                    else:
                        eng[CFG["k2"]].tensor_mul(K2[:tl, :], K, K)
                    KP[2] = K2[:tl, :]
                Q3 = pows.tile([128, H], FW, tag="Q3")
                eng[CFG["q3"]].tensor_mul(Q3[:tl, :], Q2[:tl, :], Q)
                if need_kp:
                    K3 = pows.tile([128, H], FW, tag="K3")
                    eng[CFG["k3"]].tensor_mul(K3[:tl, :], K2[:tl, :], K)
                    KP[3] = K3[:tl, :]
                QP = {1: Q, 2: Q2[:tl, :], 3: Q3[:tl, :]}
                if D >= 4:
                    Q4 = pows.tile([128, H], FW, tag="Q4")
                    nc.scalar.activation(Q4[:tl, :], Q2[:tl, :], AF.Square)
                    K4 = pows.tile([128, H], FW, tag="K4")
                    nc.scalar.activation(K4[:tl, :], K2[:tl, :], AF.Square)
                    QP[4] = Q4[:tl, :]
                    KP[4] = K4[:tl, :]

                # ---- scaled den moments s_d -> A2d (direct for non-raw)
                def den_accum(d):
                    js = scrp.tile([128, H], FW, tag="scr")
                    tgt = A2d[:tl, dcol[d] : dcol[d] + 1]
                    mode = CFG["s" + str(min(d, 3))]
                    if mode == "act":
                        nc.scalar.activation(
                            js[:tl, :], QP[d], AF.Identity,
                            scale=float(coef[d]), accum_out=tgt,
                        )
                    else:
                        nc.vector.tensor_scalar(
                            out=js[:tl, :], in0=QP[d], scalar1=float(coef[d]),
                            scalar2=0.0, op0=OP.mult, op1=OP.add,
                            accum_out=tgt,
                        )

                for d in range(1, D + 1):
                    if d in raw_ds:
                        continue
                    den_accum(d)

                # scale raw den moments by coefficients (tiny TT)
                if ncden:
                    nc.vector.tensor_mul(
                        A2d[:tl, 0:ncden], SmD[:tl, 0:ncden], cden[:tl, 0:ncden]
                    )

                # ---- num moments m_d (coef folded into op scalar/scale)
                for d in range(1, D + 1):
                    mode = CFG["m"][d - 1]
                    if mode == "ttr":
                        sd = scrp.tile([128, H], FW, tag="scr")
                        nc.vector.tensor_tensor_reduce(
                            out=sd[:tl, :], in0=QP[d], in1=X[:tl, :],
                            scale=float(coef[d]), scalar=0.0,
                            op0=OP.mult, op1=OP.add,
                            accum_out=A2n[:tl, d : d + 1],
                        )
                    elif mode == "stt":
                        sd = scrp.tile([128, H], FW, tag="scr")
                        nc.vector.scalar_tensor_tensor(
                            out=sd[:tl, :], in0=QP[d], scalar=float(coef[d]),
                            in1=X[:tl, :], op0=OP.mult, op1=OP.mult,
                            accum_out=A2n[:tl, d : d + 1],
                        )
                    else:
                        vd = scrp.tile([128, H], FW, tag=f"v{d}")
                        nc.gpsimd.tensor_mul(vd[:tl, :], QP[d], X[:tl, :])
                        jd = scrp.tile([128, H], FW, tag="scr")
                        if mode == "pool_act":
                            nc.scalar.activation(
                                jd[:tl, :], vd[:tl, :], AF.Identity,
                                scale=float(coef[d]),
                                accum_out=A2n[:tl, d : d + 1],
                            )
                        else:
                            nc.vector.tensor_scalar(
                                out=jd[:tl, :], in0=vd[:tl, :],
                                scalar1=float(coef[d]), scalar2=0.0,
                                op0=OP.mult, op1=OP.add,
                                accum_out=A2n[:tl, d : d + 1],
                            )

                return {
                    "X": X, "KP": KP, "A2n": A2n, "A2d": A2d,
                    "dcol": dcol, "aN0": aN0,
                }

        def tail(bi, st):
            t0, tl = BLOCKS[bi]
            if True:
                X, KP, A2n, A2d = st["X"], st["KP"], st["A2n"], st["A2d"]
                aN0 = st["aN0"]
                K = KP[1]

                # ---- chains in the power basis (ascending)
                def chain_horner(a, tag, last_f32=False, bias_end=None):
                    # u = a(D)*k; u = (u + a(d))*k for d=D-1..1 [; u += bias_end]
                    u = work.tile([128, H], FW, tag=f"u{tag}")
                    nc.vector.tensor_scalar(
                        out=u[:tl, :], in0=K, scalar1=a(D), scalar2=None,
                        op0=OP.mult,
                    )
                    cur = u
                    for d in range(D - 1, 0, -1):
                        last = d == 1 and bias_end is None
                        odt = F32 if (last and last_f32) else FW
                        nxt_t = work.tile([128, H], odt, tag=f"u{tag}{d}")
                        nc.vector.scalar_tensor_tensor(
                            out=nxt_t[:tl, :], in0=cur[:tl, :], scalar=a(d),
                            in1=K, op0=OP.add, op1=OP.mult,
                        )
                        cur = nxt_t
                    if bias_end is not None:
                        fin = work.tile(
                            [128, H], F32 if last_f32 else FW, tag=f"u{tag}f"
                        )
                        if CFG.get("uD_bias_act"):
                            nc.scalar.activation(
                                fin[:tl, :], cur[:tl, :], AF.Identity,
                                bias=aD0c[:tl, :],
                            )
                        else:
                            nc.vector.tensor_scalar(
                                out=fin[:tl, :], in0=cur[:tl, :],
                                scalar1=bias_end, scalar2=None, op0=OP.add,
                            )
                        cur = fin
                    return cur

                def chain(modes, a, tag, bias0=None, last_f32=False):
                    u = work.tile([128, H], FW, tag=f"u{tag}")
                    if modes[0] == "act":
                        nc.scalar.activation(
                            u[:tl, :], K, AF.Identity, scale=a(1),
                            bias=0.0 if bias0 is None else bias0,
                        )
                    elif bias0 is None:
                        nc.vector.tensor_scalar(
                            out=u[:tl, :], in0=K, scalar1=a(1),
                            scalar2=None, op0=OP.mult,
                        )
                    else:
                        nc.vector.tensor_scalar(
                            out=u[:tl, :], in0=K, scalar1=a(1),
                            scalar2=bias0, op0=OP.mult, op1=OP.add,
                        )
                    cur = u
                    for d in range(2, D + 1):
                        last = d == D
                        odt = F32 if (last and last_f32) else FW
                        mode = modes[d - 1]
                        nxt_t = work.tile([128, H], odt, tag=f"u{tag}{d}")
                        if mode == "stt":
                            nc.vector.scalar_tensor_tensor(
                                out=nxt_t[:tl, :], in0=KP[d], scalar=a(d),
                                in1=cur[:tl, :], op0=OP.mult, op1=OP.add,
                            )
                        else:
                            td = scrp.tile([128, H], FW, tag="scr")
                            nc.vector.tensor_scalar(
                                out=td[:tl, :], in0=KP[d], scalar1=a(d),
                                scalar2=None, op0=OP.mult,
                            )
                            add_eng = nc.gpsimd if mode == "ts_pool" else nc.vector
                            add_eng.tensor_add(
                                nxt_t[:tl, :], td[:tl, :], cur[:tl, :]
                            )
                        cur = nxt_t
                    return cur

                aDf = lambda d: A2d[:tl, dcol[d] : dcol[d] + 1]
                aNf = lambda d: A2n[:tl, d : d + 1]
                if CFG["uD_form"] == "horner":
                    uD = chain_horner(aDf, "d", last_f32=True, bias_end=aD0)
                else:
                    uD = chain(CFG["uD"], aDf, "d", bias0=aD0, last_f32=True)
                if CFG["uN_form"] == "horner":
                    uN = chain_horner(aNf, "n")
                else:
                    uN = chain(CFG["uN"], aNf, "n")

                # ---- out = (uN + aN0) * (1/den)
                rD = work.tile([128, H], F32, tag="rD")
                nc.vector.reciprocal_approx_fast(rD[:tl, :], uD[:tl, :])
                O = io.tile([128, H], F32, tag="O")
                if CFG["final"] == "stt":
                    nc.vector.scalar_tensor_tensor(
                        out=O[:tl, :], in0=uN[:tl, :], scalar=aN0,
                        in1=rD[:tl, :], op0=OP.add, op1=OP.mult,
                    )
                else:
                    tf = scrp.tile([128, H], FW, tag="scr")
                    nc.vector.tensor_scalar(
                        out=tf[:tl, :], in0=uN[:tl, :], scalar1=aN0,
                        scalar2=None, op0=OP.add,
                    )
                    nc.gpsimd.tensor_mul(O[:tl, :], tf[:tl, :], rD[:tl, :])
                out_eng.dma_start(out=out[t0 : t0 + tl, :], in_=O[:tl, :])

        def body():
            if CFG["interleave"]:
                s0 = head(0)
                s1 = head(1)
                tail(0, s0)
                tail(1, s1)
            else:
                for bi in range(len(BLOCKS)):
                    tail(bi, head(bi))

        if reps == 1:
            body()
        else:
            with tc.For_i(0, reps, 1):
                body()

    nc.compile()
    return nc


_NCS = {}


def _get_nc(with_bias: bool = True):
    if with_bias not in _NCS:
        _NCS[with_bias] = build_kernel(with_bias=with_bias)
    return _NCS[with_bias]


def _make_in_maps(x, W0, b0, W1, b1):
    coef = COEFS[D]
    raw_ds = [d for d, k in [(1, "s1"), (2, "s2")] if CFG[k] == "free"]
    ncden = len(raw_ds)
    npw = np.float16 if CFG["dtype"] == "f16" else np.float32
    xf = np.ascontiguousarray(np.asarray(x, np.float32).reshape(T, H))
    W0 = np.asarray(W0, np.float32).astype(npw)
    W1 = np.asarray(W1, np.float32).astype(npw)
    biasQ = np.zeros((128, H), npw)
    biasQ[0, :] = np.asarray(b1, np.float32).astype(npw)
    biasK = np.zeros((128, H), npw)
    biasK[0, :] = np.asarray(b0, np.float32).astype(npw)
    cdn = np.tile(
        np.array(
            [coef[d] for d in raw_ds] or [0.0], np.float32
        ).reshape(1, max(ncden, 1)),
        (128, 1),
    ).astype(np.float32)
    wcat = np.ascontiguousarray(
        np.concatenate(
            [W1[:128, :], W1[128:, :], biasQ, W0[:128, :], W0[128:, :], biasK],
            axis=1,
        )
    )  # [128, 6H] fp16
    maps = []
    for c in range(NCORES):
        sh = np.ascontiguousarray(xf[c * TC : (c + 1) * TC]).astype(npw)
        xst = np.ascontiguousarray(
            np.transpose(sh.reshape(TC, 2, 128), (2, 1, 0))
        )
        maps.append({"xs": sh, "xst": xst, "wcat": wcat, "cdn": cdn})
    return maps


def _ensure_axon():
    try:
        import jax
        if not any(d.platform == "axon" for d in jax.devices()):
            jax.config.update("jax_platforms", "axon,cpu")
    except Exception:
        pass


def _run(x, W0, b0, W1, b1, trace=False, **kw):
    _ensure_axon()
    with_bias = bool(
        np.any(np.asarray(b0, np.float32)) or np.any(np.asarray(b1, np.float32))
    )
    res = run_bass_kernel_spmd(
        _get_nc(with_bias), _make_in_maps(x, W0, b0, W1, b1),
        list(range(NCORES)), trace=trace, **kw,
    )
    outs = [res.results[c]["out"] for c in range(NCORES)]
    full = np.concatenate(outs, axis=0).reshape(B, S, M, H).astype(np.float32)
    return full, res


def kernel(x, W0, b0, W1, b1):
    full, _ = _run(x, W0, b0, W1, b1, trace=False)
    return full
